# revision 24
# baseline (speedup 1.0000x reference)
"""YOLO-style DetectionLoss on 8 Trainium2 NeuronCores (Bass/Tile).

Pure data parallelism: batch 8192 -> 1024 per core; 1024*7*7 = 50176
cells laid out as 128 SBUF partitions x 392 cells (each partition owns 8
consecutive images). The host concatenates output|target per cell into
one [bc, S, S, 70] tensor so each chunk needs a single DMA.

Per chunk of k cells the kernel builds masked residual tiles whose
squares sum to the loss:

  V[...,b,0:4] = resp_b*(dxy | dwh)         (xy+wh; weight 5 folded into
                                             the closing Square's scale)
  V4[...,b]    = resp_b*(pc_b - max_iou)    (contain term)
  Vc[...,c]    = obj*(pcls_c - tcls_c)      (class term)
  pcm[...,b]   = noobj*pc_b                 (noobj; 0.5 via Square scale)

Each chunk closes with ACT Square+accumulate ops -> 4 accumulator slots
per chunk, summed on the host and divided by the global batch.

The responsible-box one-hot uses reduce_max + is_equal (exact fp match);
ties can only occur when every IoU in a cell is exactly 0 (measure-zero
effect on the loss).
"""

import os

os.environ.setdefault("JAX_COMPILATION_CACHE_DIR", "/tmp/jaxcache")
os.environ.setdefault("JAX_PERSISTENT_CACHE_MIN_COMPILE_TIME_SECS", "1")
os.environ.setdefault("JAX_PERSISTENT_CACHE_MIN_ENTRY_SIZE_BYTES", "0")

import numpy as np

import concourse.bacc as bacc
import concourse.mybir as mybir
import concourse.tile as tile
from concourse.bass_utils import run_bass_kernel_spmd

F32 = mybir.dt.float32
AF = mybir.ActivationFunctionType
OP = mybir.AluOpType
AX = mybir.AxisListType

NB, C, S = 3, 20, 7
D = 5 * NB + C                 # 35
D2 = 2 * D                     # 70: output | target concatenated
B = 8192
NCORES = 8
P = 128

SQRT5 = 5.0 ** 0.5
NTERMS = 4                     # xywh, contain, class, noobj


def default_chunks(kpp):
    if kpp == 392:
        return [28, 98, 98, 98, 70]
    if kpp % 98 == 0:
        return [98] * (kpp // 98)
    if kpp % 49 == 0:
        return [49] * (kpp // 49)
    return [kpp]


def build_nc(bc: int, ks=None, repeats: int = 1, io_bufs: int = 3,
             loop_repeats: int = 0, parts: str = "full"):
    """Trace the per-core Bass program for a per-core batch of `bc`.

    The single input `cat` is [bc, S, S, 70] fp32: output channels 0:35,
    target channels 35:70 (host-concatenated).
    """
    cells = bc * S * S
    assert cells % P == 0
    kpp = cells // P
    if ks is None:
        ks = default_chunks(kpp)
    assert sum(ks) == kpp
    nchunks = len(ks)

    nc = bacc.Bacc("TRN2", debug=False, num_devices=NCORES)
    cat_h = nc.dram_tensor("cat", [bc, S, S, D2], F32, kind="ExternalInput")
    acc_h = nc.dram_tensor("acc", [P, NTERMS * nchunks], F32,
                           kind="ExternalOutput")

    cat_v = cat_h.ap().rearrange("(p a) h w d -> p (a h w d)", p=P)

    with tile.TileContext(nc) as tc:
        with (
            tc.tile_pool(name="io", bufs=io_bufs) as io_pool,
            tc.tile_pool(name="pv", bufs=2) as pv,       # V box residuals
            tc.tile_pool(name="pvc", bufs=2) as pvc,     # Vc class residuals
            tc.tile_pool(name="p6", bufs=2) as p6,       # [k,3,2] temps
            tc.tile_pool(name="pw", bufs=2) as pw_pool,  # dwt [k,3,4]
            tc.tile_pool(name="psqrt", bufs=2) as psqrt, # sp/st
            tc.tile_pool(name="p3", bufs=2) as p3,       # [k,3] temps
            tc.tile_pool(name="p1", bufs=2) as p1,       # [k] temps
            tc.tile_pool(name="accp", bufs=1) as accp,
        ):
            acc = accp.tile([P, NTERMS * nchunks], F32)

            import contextlib
            loop_cm = (tc.For_i(0, loop_repeats, 1) if loop_repeats
                       else contextlib.nullcontext())
            with loop_cm:
                for rep in range(repeats):
                    off = 0
                    pending_closings = None
                    for ci, k in enumerate(ks):
                        prev_closings = pending_closings
                        pending_closings = None
                        ct = io_pool.tile([P, k * D2], F32, name="ct", tag="ct")
                        nc.sync.dma_start(ct[:], cat_v[:, off:off + k * D2])
                        off += k * D2

                        c3 = ct[:].rearrange("p (k d) -> p k d", d=D2)
                        ob = c3[:, :, 0:15].rearrange(
                            "p k (b f) -> p k b f", f=5)
                        tb = c3[:, :, D:D + 15].rearrange(
                            "p k (b f) -> p k b f", f=5)

                        pxy = ob[:, :, :, 0:2]
                        pwh = ob[:, :, :, 2:4]
                        pc_ = ob[:, :, :, 4]
                        twh = tb[:, :, :, 2:4]
                        t0 = tb[:, :, 0, :]
                        tw0 = c3[:, :, D + 2]
                        th0 = c3[:, :, D + 3]
                        conf = c3[:, :, D + 4]
                        ocls = c3[:, :, 15:35]
                        tcls = c3[:, :, D + 15:D + 35]

                        txy0b = t0[:, :, 0:2].unsqueeze(2).broadcast_to(
                            [P, k, 3, 2])
                        twh0b = t0[:, :, 2:4].unsqueeze(2).broadcast_to(
                            [P, k, 3, 2])
                        conf3 = conf.unsqueeze(2).broadcast_to([P, k, 3])
                        conf20 = conf.unsqueeze(2).broadcast_to([P, k, 20])

                        sl = ci * NTERMS

                        # -------- tiles --------
                        V = pv.tile([P, k, 3, 5], F32, name="V", tag="V")[:]
                        Vc = pvc.tile([P, k, 20], F32, name="Vc", tag="Vc")[:]
                        dwt = pw_pool.tile([P, k, 3, 4], F32, name="dwt",
                                           tag="dwt")[:]
                        sp = psqrt.tile([P, k, 3, 2], F32, name="sp",
                                        tag="sp")[:]
                        st = psqrt.tile([P, k, 3, 2], F32, name="st",
                                        tag="st")[:]
                        dcx = p6.tile([P, k, 3, 2], F32, name="dcx",
                                      tag="dcx")[:]
                        spt = p6.tile([P, k, 3, 2], F32, name="spt",
                                      tag="spt")[:]
                        m = p6.tile([P, k, 3, 2], F32, name="m", tag="m")[:]
                        inter = p3.tile([P, k, 3], F32, name="inter",
                                        tag="inter")[:]
                        a1 = p3.tile([P, k, 3], F32, name="a1", tag="a1")[:]
                        s4 = p3.tile([P, k, 3], F32, name="s4", tag="s4")[:]
                        a24 = p1.tile([P, k], F32, name="a24", tag="a24")[:]
                        rcp = p3.tile([P, k, 3], F32, name="rcp", tag="rcp")[:]
                        miou = p1.tile([P, k], F32, name="miou", tag="miou")[:]
                        e = p3.tile([P, k, 3], F32, name="e", tag="e")[:]
                        dc = p3.tile([P, k, 3], F32, name="dc", tag="dc")[:]
                        pcm = p3.tile([P, k, 3], F32, name="pcm", tag="pcm")[:]
                        nm = p1.tile([P, k], F32, name="nm", tag="nm")[:]

                        a24b = a24.unsqueeze(2).broadcast_to([P, k, 3])
                        mioub = miou.unsqueeze(2).broadcast_to([P, k, 3])
                        nm3b = nm.unsqueeze(2).broadcast_to([P, k, 3])
                        eb4 = e.unsqueeze(3).broadcast_to([P, k, 3, 4])

                        do_box = parts in ("full", "noclass")
                        do_cls = parts in ("full", "cls")

                        if parts == "dma":
                            nc.scalar.activation(pcm, pc_, AF.Square,
                                                 accum_out=acc[:, sl:sl + 1])
                            continue

                        # -------- ACT: early unary work --------
                        if do_box:
                            nc.scalar.activation(sp, pwh, AF.Sqrt)
                            nc.scalar.activation(st, twh, AF.Sqrt)

                        # -------- DVE: nm first (Pool pcm needs it) --------
                        # nm = (conf != 1) (noobj mask; 0.5 via Square scale)
                        nc.vector.tensor_scalar(nm, conf, 1.0, None,
                                                op0=OP.not_equal)

                        # -------- Pool: no in-place writes ----------------
                        if do_box:
                            nc.gpsimd.tensor_sub(dwt[:, :, :, 0:2], pxy,
                                                 tb[:, :, :, 0:2])
                            nc.gpsimd.tensor_mul(a1, ob[:, :, :, 2],
                                                 ob[:, :, :, 3])
                            nc.gpsimd.tensor_mul(a24, tw0, th0)
                            nc.gpsimd.tensor_add(s4, a1, a24b)
                            nc.gpsimd.tensor_sub(dwt[:, :, :, 2:4], sp, st)
                        if do_cls:
                            nc.gpsimd.tensor_sub(Vc, ocls, tcls)
                        # pcm = nm * pc ((sqrt.5*pcm)^2 = .5*noobj*pc^2)
                        nc.gpsimd.tensor_mul(pcm, pc_, nm3b)

                        if not do_box:
                            def make_closings(pcm=pcm, Vc=Vc, sl=sl,
                                              do_cls=do_cls, conf20=conf20):
                                def emit():
                                    nc.scalar.activation(
                                        pcm, pcm, AF.Square, scale=0.5 ** 0.5,
                                        accum_out=acc[:, sl + 3:sl + 4])
                                    if do_cls:
                                        nc.vector.tensor_mul(Vc, Vc, conf20)
                                        nc.scalar.activation(
                                            Vc, Vc, AF.Square,
                                            accum_out=acc[:, sl + 2:sl + 3])
                                return emit
                            if prev_closings is not None:
                                prev_closings()
                            pending_closings = make_closings()
                            continue

                        # -------- DVE: IoU / responsibility chain --------
                        nc.vector.tensor_sub(dcx, pxy, txy0b)
                        nc.scalar.activation(dcx, dcx, AF.Abs, scale=2.0 / S)
                        nc.vector.tensor_add(spt, pwh, twh0b)
                        nc.vector.tensor_sub(spt, spt, dcx)      # u, in place
                        nc.vector.tensor_tensor(m, pwh, twh0b, op=OP.min)
                        nc.vector.scalar_tensor_tensor(
                            m, m, 2.0, spt, op0=OP.mult, op1=OP.min)
                        nc.scalar.activation(m, m, AF.Relu)
                        if prev_closings is not None:
                            prev_closings()
                        nc.vector.tensor_mul(inter, m[:, :, :, 0],
                                             m[:, :, :, 1])
                        nc.vector.scalar_tensor_tensor(  # den4, in place
                            s4, s4, 4.0, inter, op0=OP.mult, op1=OP.subtract)
                        nc.vector.reciprocal(rcp, s4)
                        nc.vector.tensor_mul(inter, inter, rcp)  # iou
                        nc.vector.tensor_reduce(miou, inter, axis=AX.X,
                                                op=OP.max)
                        nc.vector.tensor_tensor(e, inter, mioub,
                                                op=OP.is_equal)
                        nc.vector.tensor_mul(e, e, conf3)        # resp
                        nc.vector.tensor_mul(V[:, :, :, 0:4], dwt, eb4)
                        nc.vector.tensor_sub(dc, pc_, mioub)
                        nc.vector.tensor_mul(V[:, :, :, 4], dc, e)
                        if do_cls:
                            nc.vector.tensor_mul(Vc, Vc, conf20)

                        # -------- ACT closings: deferred one chunk --------
                        def make_closings(pcm=pcm, V=V, Vc=Vc, sl=sl,
                                          do_cls=do_cls):
                            def emit():
                                nc.scalar.activation(
                                    pcm, pcm, AF.Square, scale=0.5 ** 0.5,
                                    accum_out=acc[:, sl + 3:sl + 4])
                                nc.scalar.activation(
                                    V[:, :, :, 0:4], V[:, :, :, 0:4],
                                    AF.Square, scale=SQRT5,
                                    accum_out=acc[:, sl:sl + 1])
                                nc.scalar.activation(
                                    V[:, :, :, 4], V[:, :, :, 4], AF.Square,
                                    accum_out=acc[:, sl + 1:sl + 2])
                                if do_cls:
                                    nc.scalar.activation(
                                        Vc, Vc, AF.Square,
                                        accum_out=acc[:, sl + 2:sl + 3])
                            return emit
                        pending_closings = make_closings()

                    if pending_closings is not None:
                        pending_closings()
                        pending_closings = None

            nc.sync.dma_start(acc_h.ap()[:], acc[:])

    nc.compile()
    return nc


_CACHE = {}


def _get_nc(bc, ks=None, repeats=1, io_bufs=3, loop_repeats=0, **kw):
    key = (bc, tuple(ks) if ks else None, repeats, io_bufs, loop_repeats,
           tuple(sorted(kw.items())))
    if key not in _CACHE:
        _CACHE[key] = build_nc(bc, ks, repeats, io_bufs, loop_repeats, **kw)
    return _CACHE[key]


def combine_acc(acc_list, nchunks):
    tot = 0.0
    for a in acc_list:
        tot += a.astype(np.float64).sum()
    return np.float32(tot / B)


def host_concat(output, target):
    """[B,S,S,35]x2 -> [B,S,S,70] output|target per cell."""
    return np.concatenate([output, target], axis=-1)


BEST_KS = [28, 98, 98, 98, 70]
BEST_IO_BUFS = 3


def kernel(output: np.ndarray, target: np.ndarray) -> np.ndarray:
    assert output.shape == (B, S, S, D) and target.shape == (B, S, S, D)
    bc = B // NCORES
    nchunks = len(BEST_KS)
    nc = _get_nc(bc, BEST_KS, io_bufs=BEST_IO_BUFS)
    cat = host_concat(output, target)
    in_maps = [
        {"cat": np.ascontiguousarray(cat[i * bc:(i + 1) * bc])}
        for i in range(NCORES)
    ]
    res = run_bass_kernel_spmd(nc, in_maps, list(range(NCORES)))
    return combine_acc([r["acc"] for r in res.results], nchunks)


# revision 37
# speedup vs baseline: 1.3165x; 1.3165x over previous
"""YOLO-style DetectionLoss on 8 Trainium2 NeuronCores (Bass/Tile).

Pure data parallelism: batch 8192 -> 1024 per core; 1024*7*7 = 50176
cells laid out as 128 SBUF partitions x 392 cells (each partition owns 8
consecutive images). The host concatenates output|target per cell into
one [bc, S, S, 70] tensor so each chunk needs a single DMA.

Per chunk of k cells the kernel builds masked residual tiles whose
squares sum to the loss:

  V[...,b,0:4] = resp_b*(dxy | dwh)         (xy+wh; weight 5 folded into
                                             the closing Square's scale)
  V4[...,b]    = resp_b*(pc_b - max_iou)    (contain term)
  Vc[...,c]    = obj*(pcls_c - tcls_c)      (class term)
  pcm[...,b]   = noobj*pc_b                 (noobj; 0.5 via Square scale)

Each chunk closes with ACT Square+accumulate ops -> 4 accumulator slots
per chunk, summed on the host and divided by the global batch.

The responsible-box one-hot uses reduce_max + is_equal (exact fp match);
ties can only occur when every IoU in a cell is exactly 0 (measure-zero
effect on the loss).
"""

import os

os.environ.setdefault("JAX_COMPILATION_CACHE_DIR", "/tmp/jaxcache")
os.environ.setdefault("JAX_PERSISTENT_CACHE_MIN_COMPILE_TIME_SECS", "1")
os.environ.setdefault("JAX_PERSISTENT_CACHE_MIN_ENTRY_SIZE_BYTES", "0")

import numpy as np

import concourse.bacc as bacc
import concourse.mybir as mybir
import concourse.tile as tile
from concourse.bass_utils import run_bass_kernel_spmd

F32 = mybir.dt.float32
BF16 = mybir.dt.bfloat16
AF = mybir.ActivationFunctionType
OP = mybir.AluOpType
AX = mybir.AxisListType

NB, C, S = 3, 20, 7
D = 5 * NB + C                 # 35
D2 = 2 * D                     # 70: output | target concatenated
B = 8192
NCORES = 8
P = 128

SQRT5 = 5.0 ** 0.5
NTERMS = 1                     # single fused residual slot


def default_chunks(kpp):
    if kpp == 392:
        return [28, 98, 98, 98, 70]
    if kpp % 98 == 0:
        return [98] * (kpp // 98)
    if kpp % 49 == 0:
        return [49] * (kpp // 49)
    return [kpp]


def build_nc(bc: int, ks=None, repeats: int = 1, io_bufs: int = 3,
             loop_repeats: int = 0, parts: str = "full",
             cls_pool: bool = False, abs_dve: bool = False,
             relu_dve: bool = False, cls_split: int = 10,
             areas_dve: bool = True, dve_all: bool = True,
             pcm_pool: bool = False, cls_sub_pool: bool = False,
             in_bf16: bool = True):
    """Trace the per-core Bass program for a per-core batch of `bc`.

    The single input `cat` is [bc, S, S, 70] fp32: output channels 0:35,
    target channels 35:70 (host-concatenated).
    """
    cells = bc * S * S
    assert cells % P == 0
    kpp = cells // P
    if ks is None:
        ks = default_chunks(kpp)
    assert sum(ks) == kpp
    nchunks = len(ks)

    nc = bacc.Bacc("TRN2", debug=False, num_devices=NCORES)
    IN_DT = BF16 if in_bf16 else F32
    cat_h = nc.dram_tensor("cat", [bc, S, S, D2], IN_DT, kind="ExternalInput")
    acc_h = nc.dram_tensor("acc", [P, NTERMS * nchunks], F32,
                           kind="ExternalOutput")

    cat_v = cat_h.ap().rearrange("(p a) h w d -> p (a h w d)", p=P)

    with tile.TileContext(nc) as tc:
        with (
            tc.tile_pool(name="io", bufs=io_bufs) as io_pool,
            tc.tile_pool(name="pv", bufs=2) as pv,       # V box residuals
            tc.tile_pool(name="pvc", bufs=2) as pvc,     # Vc class residuals
            tc.tile_pool(name="p6", bufs=2) as p6,       # [k,3,2] temps
            tc.tile_pool(name="pw", bufs=2) as pw_pool,  # dwt [k,3,4]
            tc.tile_pool(name="psqrt", bufs=2) as psqrt, # sp/st
            tc.tile_pool(name="p3", bufs=2) as p3,       # [k,3] temps
            tc.tile_pool(name="p1", bufs=2) as p1,       # [k] temps
            tc.tile_pool(name="accp", bufs=1) as accp,
        ):
            acc = accp.tile([P, NTERMS * nchunks], F32)

            import contextlib
            loop_cm = (tc.For_i(0, loop_repeats, 1) if loop_repeats
                       else contextlib.nullcontext())
            with loop_cm:
                for rep in range(repeats):
                    off = 0
                    pending_closings = None
                    for ci, k in enumerate(ks):
                        prev_closings = pending_closings
                        pending_closings = None
                        ct = io_pool.tile([P, k * D2], IN_DT, name="ct",
                                          tag="ct")
                        nc.sync.dma_start(ct[:], cat_v[:, off:off + k * D2])
                        off += k * D2

                        c3 = ct[:].rearrange("p (k d) -> p k d", d=D2)
                        ob = c3[:, :, 0:15].rearrange(
                            "p k (b f) -> p k b f", f=5)
                        tb = c3[:, :, D:D + 15].rearrange(
                            "p k (b f) -> p k b f", f=5)

                        pxy = ob[:, :, :, 0:2]
                        pwh = ob[:, :, :, 2:4]
                        pc_ = ob[:, :, :, 4]
                        twh = tb[:, :, :, 2:4]
                        t0 = tb[:, :, 0, :]
                        tw0 = c3[:, :, D + 2]
                        th0 = c3[:, :, D + 3]
                        conf = c3[:, :, D + 4]
                        ocls = c3[:, :, 15:35]
                        tcls = c3[:, :, D + 15:D + 35]

                        txy0b = t0[:, :, 0:2].unsqueeze(2).broadcast_to(
                            [P, k, 3, 2])
                        twh0b = t0[:, :, 2:4].unsqueeze(2).broadcast_to(
                            [P, k, 3, 2])
                        conf3 = conf.unsqueeze(2).broadcast_to([P, k, 3])
                        conf20 = conf.unsqueeze(2).broadcast_to([P, k, 20])

                        sl = ci * NTERMS

                        # -------- tiles --------
                        R = pv.tile([P, k, 38], F32, name="R", tag="R")[:]
                        V = R[:, :, 0:15].rearrange("p k (b f) -> p k b f", f=5)
                        pcmv = R[:, :, 15:18]
                        Vc = R[:, :, 18:38]
                        Vc2 = Vc
                        dwt = pw_pool.tile([P, k, 3, 4], F32, name="dwt",
                                           tag="dwt")[:]
                        sp = psqrt.tile([P, k, 3, 2], F32, name="sp",
                                        tag="sp")[:]
                        st = psqrt.tile([P, k, 3, 2], F32, name="st",
                                        tag="st")[:]
                        dcx = p6.tile([P, k, 3, 2], F32, name="dcx",
                                      tag="dcx")[:]
                        spt = p6.tile([P, k, 3, 2], F32, name="spt",
                                      tag="spt")[:]
                        m = p6.tile([P, k, 3, 2], F32, name="m", tag="m")[:]
                        inter = p3.tile([P, k, 3], F32, name="inter",
                                        tag="inter")[:]
                        a1 = p3.tile([P, k, 3], F32, name="a1", tag="a1")[:]
                        s4 = p3.tile([P, k, 3], F32, name="s4", tag="s4")[:]
                        a24 = p1.tile([P, k], F32, name="a24", tag="a24")[:]
                        rcp = p3.tile([P, k, 3], F32, name="rcp", tag="rcp")[:]
                        miou = p1.tile([P, k], F32, name="miou", tag="miou")[:]
                        e = p3.tile([P, k, 3], F32, name="e", tag="e")[:]
                        dc = p3.tile([P, k, 3], F32, name="dc", tag="dc")[:]
                        pcm = pcmv
                        em5 = p3.tile([P, k, 3], F32, name="em5", tag="em5")[:]
                        nm = p1.tile([P, k], F32, name="nm", tag="nm")[:]

                        a24b = a24.unsqueeze(2).broadcast_to([P, k, 3])
                        mioub = miou.unsqueeze(2).broadcast_to([P, k, 3])
                        nm3b = nm.unsqueeze(2).broadcast_to([P, k, 3])
                        em5b4 = em5.unsqueeze(3).broadcast_to([P, k, 3, 4])

                        do_box = parts in ("full", "noclass")
                        do_cls = parts in ("full", "cls")

                        if parts == "dma":
                            nc.scalar.activation(pcm, pc_, AF.Square,
                                                 accum_out=acc[:, sl:sl + 1])
                            continue

                        # -------- ACT: early unary work --------
                        if do_box:
                            nc.scalar.activation(sp, pwh, AF.Sqrt)
                            nc.scalar.activation(st, twh, AF.Sqrt)

                        # -------- ACT: nm = sqrt(.5)*(1-conf), exact for
                        # conf in {0,1} (noobj mask with weight folded)
                        nc.scalar.activation(nm, conf, AF.Copy,
                                             scale=-(0.5 ** 0.5),
                                             bias=0.5 ** 0.5)

                        # -------- Pool: no in-place writes ----------------
                        eng = nc.vector if dve_all else nc.gpsimd
                        if do_box:
                            eng.tensor_sub(dwt[:, :, :, 0:2], pxy,
                                           tb[:, :, :, 0:2])
                            if not areas_dve:
                                nc.gpsimd.tensor_mul(a1, ob[:, :, :, 2],
                                                     ob[:, :, :, 3])
                                nc.gpsimd.tensor_mul(a24, tw0, th0)
                                nc.gpsimd.tensor_add(s4, a1, a24b)
                            eng.tensor_sub(dwt[:, :, :, 2:4], sp, st)
                        if do_cls:
                            (nc.gpsimd if cls_sub_pool else eng).tensor_sub(
                                Vc, ocls, tcls)
                            if cls_pool:
                                h = cls_split
                                nc.gpsimd.tensor_mul(
                                    Vc2[:, :, 0:h], Vc[:, :, 0:h],
                                    conf.unsqueeze(2).broadcast_to([P, k, h]))
                        # pcm = nm * pc ((sqrt.5*pcm)^2 = .5*noobj*pc^2)
                        (nc.gpsimd if pcm_pool else eng).tensor_mul(
                            pcm, pc_, nm3b)

                        if not do_box:
                            def make_closings(pcm=pcm, Vc=Vc, sl=sl,
                                              do_cls=do_cls, conf20=conf20):
                                def emit():
                                    nc.scalar.activation(
                                        pcm, pcm, AF.Square, scale=0.5 ** 0.5,
                                        accum_out=acc[:, sl:sl + 1])
                                    if do_cls:
                                        nc.vector.tensor_mul(Vc, Vc, conf20)
                                        nc.scalar.activation(
                                            Vc, Vc, AF.Square,
                                            accum_out=acc[:, sl:sl + 1])
                                return emit
                            if prev_closings is not None:
                                prev_closings()
                            pending_closings = make_closings()
                            continue

                        # -------- DVE: IoU / responsibility chain --------
                        if areas_dve:
                            nc.vector.tensor_mul(a1, ob[:, :, :, 2],
                                                 ob[:, :, :, 3])
                            nc.vector.tensor_mul(a24, tw0, th0)
                            nc.vector.tensor_add(s4, a1, a24b)
                        nc.vector.tensor_sub(dcx, pxy, txy0b)
                        nc.vector.tensor_add(spt, pwh, twh0b)
                        if abs_dve:
                            # |dcx| on DVE; u = spt - (2/S)*|dcx| via stt
                            nc.vector.tensor_tensor(dcx, dcx, dcx,
                                                    op=OP.abs_max)
                            nc.vector.scalar_tensor_tensor(
                                spt, dcx, -2.0 / S, spt,
                                op0=OP.mult, op1=OP.add)  # u, in place
                        else:
                            nc.scalar.activation(dcx, dcx, AF.Abs,
                                                 scale=2.0 / S)
                            nc.vector.tensor_sub(spt, spt, dcx)  # u, in place
                        nc.vector.tensor_tensor(m, pwh, twh0b, op=OP.min)
                        nc.vector.scalar_tensor_tensor(
                            m, m, 2.0, spt, op0=OP.mult, op1=OP.min)
                        if relu_dve:
                            nc.vector.tensor_scalar_max(m, m, 0.0)
                        else:
                            nc.scalar.activation(m, m, AF.Relu)
                        if prev_closings is not None:
                            prev_closings()
                        nc.vector.tensor_mul(inter, m[:, :, :, 0],
                                             m[:, :, :, 1])
                        nc.vector.scalar_tensor_tensor(  # den4, in place
                            s4, s4, 4.0, inter, op0=OP.mult, op1=OP.subtract)
                        nc.vector.reciprocal(rcp, s4)
                        nc.vector.tensor_mul(inter, inter, rcp)  # iou
                        nc.vector.tensor_reduce(miou, inter, axis=AX.X,
                                                op=OP.max)
                        nc.vector.tensor_tensor(e, inter, mioub,
                                                op=OP.is_equal)
                        nc.vector.tensor_mul(e, e, conf3)        # resp
                        nc.scalar.activation(em5, e, AF.Copy, scale=SQRT5)
                        nc.vector.tensor_mul(V[:, :, :, 0:4], dwt, em5b4)
                        nc.vector.tensor_sub(dc, pc_, mioub)
                        nc.vector.tensor_mul(V[:, :, :, 4], dc, e)
                        if do_cls:
                            if cls_pool:
                                h = cls_split
                                if h < 20:
                                    nc.vector.tensor_mul(
                                        Vc2[:, :, h:20], Vc[:, :, h:20],
                                        conf.unsqueeze(2).broadcast_to(
                                            [P, k, 20 - h]))
                            else:
                                nc.vector.tensor_mul(Vc, Vc, conf20)

                        # -------- ACT closing: deferred one chunk --------
                        def make_closings(R=R, sl=sl):
                            def emit():
                                nc.scalar.activation(
                                    R, R, AF.Square,
                                    accum_out=acc[:, sl:sl + 1])
                            return emit
                        pending_closings = make_closings()

                    if pending_closings is not None:
                        pending_closings()
                        pending_closings = None

            nc.sync.dma_start(acc_h.ap()[:], acc[:])

    nc.compile()
    return nc


_CACHE = {}


def _get_nc(bc, ks=None, repeats=1, io_bufs=3, loop_repeats=0, **kw):
    key = (bc, tuple(ks) if ks else None, repeats, io_bufs, loop_repeats,
           tuple(sorted(kw.items())))
    if key not in _CACHE:
        _CACHE[key] = build_nc(bc, ks, repeats, io_bufs, loop_repeats, **kw)
    return _CACHE[key]


def combine_acc(acc_list, nchunks):
    tot = 0.0
    for a in acc_list:
        tot += a.astype(np.float64).sum()
    return np.float32(tot / B)


def host_concat(output, target, bf16=True):
    """[B,S,S,35]x2 -> [B,S,S,70] output|target per cell."""
    cat = np.concatenate([output, target], axis=-1)
    if bf16:
        import ml_dtypes
        cat = cat.astype(ml_dtypes.bfloat16)
    return cat


BEST_KS = [49, 147, 147, 49]
BEST_IO_BUFS = 3


def kernel(output: np.ndarray, target: np.ndarray) -> np.ndarray:
    assert output.shape == (B, S, S, D) and target.shape == (B, S, S, D)
    bc = B // NCORES
    nchunks = len(BEST_KS)
    nc = _get_nc(bc, BEST_KS, io_bufs=BEST_IO_BUFS)
    cat = host_concat(output, target)
    in_maps = [
        {"cat": np.ascontiguousarray(cat[i * bc:(i + 1) * bc])}
        for i in range(NCORES)
    ]
    res = run_bass_kernel_spmd(nc, in_maps, list(range(NCORES)))
    return combine_acc([r["acc"] for r in res.results], nchunks)
